# revision 24
# baseline (speedup 1.0000x reference)
"""Trainium2 Bass kernel for nn_EBlock (GNN message passing).

Strategy (8 NeuronCores, SPMD single program):
  * Edges are partitioned by DESTINATION node range (host-side sort), so the
    scatter-sum stays core-local -- no AllReduce of [N, HID] partials.
  * Node projection is shard-computed and AllGathered as a f16 gather table.
  * Per-edge gather hv[src] is done with dma_gather (int16 indices) straight
    from the AllGather output, with the table split in two halves (lo/hi)
    because indices are int16.
  * The segment sum uses the sorted one-hot matmul trick: per 128-dst "bin",
    S[e, w] = (slot[e] == w) and h_bin += msgs_tile^T @ S_tile accumulated in
    PSUM on the TensorEngine.
  * LayerNorm mean-centering of the edge projection is folded into the weight
    matrix (W' = W - rowmean(W) x 1), so y' = x @ W' is already centered and
    the only per-edge statistic needed is q = sum(y'^2) = x^T (W'W'^T) x.
    q is computed in a separate stats pass (overlapping the AllGather) on
    PE+DVE via the A-matrix trick, and ln/exp for rstd run ONCE batched over
    all bins -- this avoids per-bin activation-table thrash.
  * rstd = exp(-0.5 * ln(var + eps)) so only the ln/exp table set is used in
    the steady state.
"""

import os
import sys

sys.path.insert(0, "/opt/trn_rl_repo")

import numpy as np
import ml_dtypes

import concourse.bass as bass
import concourse.bacc as bacc
import concourse.mybir as mybir
import concourse.tile as tile
from concourse.tile import add_dep_helper
from concourse.bass_utils import run_bass_kernel_spmd

F16 = np.float16

# tiles per dma_gather call (4 => 512 idxs; >4 risks the HW packet limit)
GSTEP = 4
# bins of he production run ahead of the gather-dependent consumer
LAG = 10

# ---------------------------------------------------------------- config

class Cfg:
    def __init__(self, n_nodes=50000, n_edges=800000, node_in=256, edge_in=64,
                 hid=128, out=16, n_cores=8, lo=32768, eps=1e-5):
        self.N, self.E = n_nodes, n_edges
        self.NODE_IN, self.EDGE_IN, self.HID, self.OUT = node_in, edge_in, hid, out
        self.NC = n_cores
        self.EPS = eps
        self.NPC = (n_nodes + n_cores - 1) // n_cores        # nodes per core
        self.NB = (self.NPC + 127) // 128                     # dst bins per core
        self.NPAD = self.NB * 128                             # padded shard rows
        self.AGROWS = self.NC * self.NPAD                     # allgather table rows
        self.LO = min(lo, self.AGROWS)                        # lo table rows
        self.HIR = self.AGROWS - self.LO                      # hi table rows
        assert self.LO <= 32768 and self.HIR <= 32768
        # K_LO / K_HI / TPB / ETOT set by prep()
        self.K_LO = self.K_HI = self.TPB = self.ETOT = None

    def key(self):
        return (self.N, self.E, self.NODE_IN, self.EDGE_IN, self.HID, self.OUT,
                self.NC, self.LO, self.K_LO, self.K_HI, GSTEP, LAG)


# ---------------------------------------------------------------- host prep

def _to_f16(x):
    return np.asarray(x, dtype=np.float32).astype(F16)


def prep(cfg, node_feats, edge_feats, src, dst,
         W_node, g_node, b_node, W_edge, g_edge, b_edge, W_out, g_out, b_out):
    """Shard/sort/pad the inputs.  Returns (in_maps, meta)."""
    N, E, NC = cfg.N, cfg.E, cfg.NC
    NPC, NB = cfg.NPC, cfg.NB
    HID, EIN, NIN, OUT = cfg.HID, cfg.EDGE_IN, cfg.NODE_IN, cfg.OUT

    src = np.asarray(src).astype(np.int64)
    dst = np.asarray(dst).astype(np.int64)
    node_feats = np.asarray(node_feats, dtype=np.float32)
    edge_feats = np.asarray(edge_feats, dtype=np.float32)

    # position of node i's hv row in the allgathered table
    src_remap = (src // NPC) * cfg.NPAD + (src % NPC)
    is_lo = src_remap < cfg.LO
    core_of_edge = dst // NPC

    percore = []
    for c in range(NC):
        sel = np.nonzero(core_of_edge == c)[0]
        d_loc = (dst[sel] - c * NPC).astype(np.int64)
        lo_cnt = np.bincount(d_loc[is_lo[sel]], minlength=NPC)
        hi_cnt = np.bincount(d_loc[~is_lo[sel]], minlength=NPC)

        # --- bin packing: NB bins of <=128 dst, balancing lo & hi loads
        order = np.argsort(-(lo_cnt + hi_cnt), kind="stable")
        bin_lo = np.zeros(NB); bin_hi = np.zeros(NB)
        bin_n = np.zeros(NB, np.int64)
        assign = np.full(NPC, -1, np.int64)
        slot = np.full(NPC, -1, np.int64)
        t_lo = max(lo_cnt.sum() / NB, 1.0)
        t_hi = max(hi_cnt.sum() / NB, 1.0)
        for d in order:
            cost = np.maximum((bin_lo + lo_cnt[d]) / t_lo,
                              (bin_hi + hi_cnt[d]) / t_hi)
            cost[bin_n >= 128] = np.inf
            b = int(np.argmin(cost))
            assign[d] = b
            slot[d] = bin_n[b]
            bin_n[b] += 1
            bin_lo[b] += lo_cnt[d]
            bin_hi[b] += hi_cnt[d]
        percore.append((sel, d_loc, assign, slot))

    # global tile counts (shared SPMD schedule)
    k_lo = k_hi = 1
    for c in range(NC):
        sel, d_loc, assign, slot = percore[c]
        lo_e = is_lo[sel]
        bin_of_edge = assign[d_loc]
        blc = np.bincount(bin_of_edge[lo_e], minlength=NB)
        bhc = np.bincount(bin_of_edge[~lo_e], minlength=NB)
        k_lo = max(k_lo, int(np.max((blc + 127) // 128)) if blc.size else 1)
        k_hi = max(k_hi, int(np.max((bhc + 127) // 128)) if bhc.size else 1)
    cfg.K_LO, cfg.K_HI = k_lo, k_hi
    cfg.TPB = k_lo + k_hi
    cfg.ETOT = NB * cfg.TPB * 128
    TPB, ETOT = cfg.TPB, cfg.ETOT

    # --- uniformity of gains/biases
    def uni(v):
        v = np.asarray(v, np.float32)
        return (float(v.flat[0]), True) if np.all(v == v.flat[0]) else (0.0, False)
    g_nu, node_g_uni = uni(g_node); b_nu, node_b_uni = uni(b_node)
    g_eu, edge_g_uni = uni(g_edge); b_eu, edge_b_uni = uni(b_edge)
    g_ou, out_g_uni = uni(g_out);  b_ou, out_b_uni = uni(b_out)

    meta = dict(g_nu=g_nu, b_nu=b_nu, g_eu=g_eu, b_eu=b_eu, g_ou=g_ou, b_ou=b_ou,
                node_uni=node_g_uni and node_b_uni,
                edge_uni=edge_g_uni and edge_b_uni,
                out_uni=out_g_uni and out_b_uni,
                inv=[])

    # --- shared weight arrays
    W_node = np.asarray(W_node, np.float32)
    W_edge = np.asarray(W_edge, np.float32)
    W_out = np.asarray(W_out, np.float32)
    assert NIN % 128 == 0
    KN = NIN // 128
    w_node_arr = np.ascontiguousarray(
        W_node.reshape(KN, 128, HID).transpose(1, 0, 2).reshape(128, KN * HID)
    ).astype(F16)
    # LN mean-centering folded into the edge weights: y' = x @ W' is centered
    W_edge_c = W_edge - W_edge.mean(axis=1, keepdims=True)
    w_edge_arr = _to_f16(W_edge_c)
    a_mat = ((W_edge_c.astype(np.float64) @ W_edge_c.astype(np.float64).T) / HID)
    a_arr = a_mat.astype(np.float32).astype(F16)
    w_out_arr = np.asarray(W_out, np.float32)
    iota_arr = np.broadcast_to(
        np.tile(np.arange(128, dtype=np.float32), TPB)[None, :], (128, TPB * 128)
    ).astype(F16)
    g_edge_rep = np.broadcast_to(np.asarray(g_edge, np.float32)[None, :], (128, HID)).copy()
    b_edge_rep = np.broadcast_to(np.asarray(b_edge, np.float32)[None, :], (128, HID)).copy()
    g_node_rep = np.broadcast_to(np.asarray(g_node, np.float32)[None, :], (128, HID)).copy()
    b_node_rep = np.broadcast_to(np.asarray(b_node, np.float32)[None, :], (128, HID)).copy()
    g_out_rep = np.broadcast_to(np.asarray(g_out, np.float32)[None, :], (128, OUT)).copy()
    b_out_rep = np.broadcast_to(np.asarray(b_out, np.float32)[None, :], (128, OUT)).copy()

    in_maps = []
    for c in range(NC):
        sel, d_loc, assign, slot = percore[c]
        lo_e = is_lo[sel]
        bin_of_edge = assign[d_loc]
        slot_of_edge = slot[d_loc]

        # position of each real edge in the padded per-core stream
        ord_e = np.lexsort((src_remap[sel], (~lo_e).astype(np.int64), bin_of_edge))
        sel_o = sel[ord_e]
        bins_o = bin_of_edge[ord_e]
        lo_o = lo_e[ord_e]
        slot_o = slot_of_edge[ord_e]
        # rank within (bin, lo/hi) group
        grp = bins_o * 2 + (~lo_o).astype(np.int64)
        # edges are sorted by grp; rank = index - first index of grp
        first = np.zeros(2 * NB, np.int64)
        cnts = np.bincount(grp, minlength=2 * NB)
        np.cumsum(cnts[:-1], out=first[1:])
        rank = np.arange(len(grp)) - first[grp]
        base = bins_o * (TPB * 128) + np.where(lo_o, 0, k_lo * 128)
        pos = base + rank
        assert len(np.unique(pos)) == len(pos)

        ef_pad = np.zeros((ETOT, EIN), np.float32)
        ef_pad[pos] = edge_feats[sel_o]
        idx_pad = np.zeros(ETOT, np.int64)
        idx_pad[pos] = np.where(lo_o, src_remap[sel_o], src_remap[sel_o] - cfg.LO)
        slot_pad = np.full(ETOT, -1.0, np.float32)
        slot_pad[pos] = slot_o.astype(np.float32)

        edge_T = np.ascontiguousarray(ef_pad.T).astype(F16)
        x_rows = np.ascontiguousarray(
            ef_pad.reshape(NB * TPB, 128, EIN).transpose(1, 0, 2).reshape(128, NB * TPB * EIN)
        ).astype(F16)
        idx16 = idx_pad.astype(np.int16).reshape(ETOT // 16, 16).T  # [16, ETOT/16]
        src_w = np.ascontiguousarray(np.tile(idx16, (8, 1)))
        dst_sl = np.ascontiguousarray(
            slot_pad.reshape(NB * TPB, 128).T
        ).astype(F16)

        nshard = np.zeros((cfg.NPAD, NIN), np.float32)
        hi = min((c + 1) * NPC, N)
        nshard[: hi - c * NPC] = node_feats[c * NPC: hi]
        node_T = np.ascontiguousarray(nshard.T).astype(F16)

        in_maps.append({
            "edge_T": edge_T, "x_rows": x_rows, "src_w": src_w, "dst_sl": dst_sl,
            "node_T": node_T, "w_node": w_node_arr, "w_edge": w_edge_arr,
            "a_mat": a_arr, "w_out": w_out_arr, "iota_in": iota_arr,
            "g_edge_rep": g_edge_rep, "b_edge_rep": b_edge_rep,
            "g_node_rep": g_node_rep, "b_node_rep": b_node_rep,
            "g_out_rep": g_out_rep, "b_out_rep": b_out_rep,
        })

        # output row of local dst d = assign[d]*128 + slot[d]
        real = np.arange(min(NPC, N - c * NPC))
        meta["inv"].append(assign[real] * 128 + slot[real])

    return in_maps, meta


# ---------------------------------------------------------------- device program

def build(cfg, meta):
    NB, TPB, K_LO, K_HI = cfg.NB, cfg.TPB, cfg.K_LO, cfg.K_HI
    HID, EIN, NIN, OUT = cfg.HID, cfg.EDGE_IN, cfg.NODE_IN, cfg.OUT
    ETOT, NPAD, AGROWS, LO = cfg.ETOT, cfg.NPAD, cfg.AGROWS, cfg.LO
    KN = NIN // 128
    EPS = cfg.EPS
    dt = mybir.dt
    f32, f16, i16 = dt.float32, dt.float16, dt.int16
    AX = mybir.AxisListType
    OP = mybir.AluOpType
    AF = mybir.ActivationFunctionType

    nc = bacc.Bacc("TRN2", target_bir_lowering=False, debug=False,
                   num_devices=cfg.NC)

    # register EPS as a usable constant bias AP for nc.scalar.activation
    _t = nc.alloc_sbuf_tensor(f"const-f32-eps", [128, 1], f32)
    nc.gpsimd.memset(_t.ap(), EPS)
    nc.const_aps.aps[(f32, EPS)] = _t.ap()
    nc.all_engine_barrier()

    def din(name, shape, d):
        return nc.dram_tensor(name, shape, d, kind="ExternalInput").ap()

    edge_T = din("edge_T", [EIN, ETOT], f16)
    x_rows = din("x_rows", [128, NB * TPB * EIN], f16)
    src_w = din("src_w", [128, ETOT // 16], i16)
    dst_sl = din("dst_sl", [128, NB * TPB], f16)
    node_T = din("node_T", [NIN, NPAD], f16)
    w_node = din("w_node", [128, KN * HID], f16)
    w_edge = din("w_edge", [EIN, HID], f16)
    a_mat = din("a_mat", [EIN, EIN], f16)
    w_out = din("w_out", [HID, OUT], f32)
    iota_in = din("iota_in", [128, TPB * 128], f16)
    g_edge_rep = din("g_edge_rep", [128, HID], f32)
    b_edge_rep = din("b_edge_rep", [128, HID], f32)
    g_node_rep = din("g_node_rep", [128, HID], f32)
    b_node_rep = din("b_node_rep", [128, HID], f32)
    g_out_rep = din("g_out_rep", [128, OUT], f32)
    b_out_rep = din("b_out_rep", [128, OUT], f32)
    out_ext = nc.dram_tensor("out", [NB * 128, OUT], f32, kind="ExternalOutput").ap()

    hv_in = nc.dram_tensor("hv_in", [NPAD, HID], f16).ap()
    hv_ag = nc.dram_tensor("hv_ag", [AGROWS, HID], f16, addr_space="Shared").ap()

    g_nu, b_nu = meta["g_nu"], meta["b_nu"]
    g_eu, b_eu = meta["g_eu"], meta["b_eu"]
    g_ou, b_ou = meta["g_ou"], meta["b_ou"]

    with tile.TileContext(nc) as tc:
        cpool = tc.alloc_tile_pool(name="consts", bufs=1)
        ppool = tc.alloc_tile_pool(name="persist", bufs=1)
        spool = tc.alloc_tile_pool(name="stats", bufs=2)
        wkpool = tc.alloc_tile_pool(name="work", bufs=2)
        ntpool = tc.alloc_tile_pool(name="nt", bufs=3)
        e1pool = tc.alloc_tile_pool(name="e1", bufs=2)
        gpool = tc.alloc_tile_pool(name="gath", bufs=2)
        hepool = tc.alloc_tile_pool(name="he", bufs=LAG + 2)
        pspool = tc.alloc_tile_pool(name="ps", bufs=3, space="PSUM")
        t1pool = tc.alloc_tile_pool(name="t1", bufs=2, space="PSUM")
        hbpool = tc.alloc_tile_pool(name="hb", bufs=2, space="PSUM")

        # ---- constants into SBUF
        wnode_sb = cpool.tile([128, KN, HID], f16)
        nc.sync.dma_start(out=wnode_sb[:], in_=w_node[:])
        wedge_sb = cpool.tile([EIN, HID], f16)
        nc.sync.dma_start(out=wedge_sb[:], in_=w_edge[:])
        amat_sb = cpool.tile([EIN, EIN], f16)
        nc.sync.dma_start(out=amat_sb[:], in_=a_mat[:])
        wout_sb = cpool.tile([HID, OUT], f32)
        nc.sync.dma_start(out=wout_sb[:], in_=w_out[:])
        iota_sb = cpool.tile([128, TPB, 128], f16)
        nc.sync.dma_start(out=iota_sb[:], in_=iota_in[:])
        srcw_sb = cpool.tile([128, ETOT // 16], i16)
        nc.sync.dma_start(out=srcw_sb[:], in_=src_w[:])
        dst_sb = cpool.tile([128, NB * TPB], f16)
        nc.sync.dma_start(out=dst_sb[:], in_=dst_sl[:])
        if not meta["edge_uni"]:
            ger_sb = cpool.tile([128, HID], f32)
            nc.sync.dma_start(out=ger_sb[:], in_=g_edge_rep[:])
            ber_sb = cpool.tile([128, HID], f32)
            nc.sync.dma_start(out=ber_sb[:], in_=b_edge_rep[:])
        if not meta["node_uni"]:
            gnr_sb = cpool.tile([128, HID], f32)
            nc.sync.dma_start(out=gnr_sb[:], in_=g_node_rep[:])
            bnr_sb = cpool.tile([128, HID], f32)
            nc.sync.dma_start(out=bnr_sb[:], in_=b_node_rep[:])
        if not meta["out_uni"]:
            gor_sb = cpool.tile([128, OUT], f32)
            nc.sync.dma_start(out=gor_sb[:], in_=g_out_rep[:])
            bor_sb = cpool.tile([128, OUT], f32)
            nc.sync.dma_start(out=bor_sb[:], in_=b_out_rep[:])

        # =================================================== phase N: hv
        g_all = ppool.tile([128, NB * HID], f16, tag="g_all")
        ex2_n = spool.tile([128, NB], f32, tag="ex2n")
        sum_n = spool.tile([128, NB], f32, tag="sumn")
        node_r = node_T.rearrange("(a p) m -> p a m", p=128)
        NTG = 4                       # node tiles per DMA (HWDGE is 625ns/copy)
        for t0 in range(0, NB, NTG):
            tn = min(NTG, NB - t0)
            nt = ntpool.tile([128, KN, NTG * 128], f16, tag="nt")
            # Activation-engine HWDGE queue: don't let the big pass-1 edge
            # loads on the SP queue starve the node-phase loads (the AllGather
            # can't start until phase N flushes).
            nc.scalar.dma_start(out=nt[:, :, :tn * 128],
                                in_=node_r[:, :, t0 * 128:(t0 + tn) * 128])
            for j in range(tn):
                t = t0 + j
                ps = pspool.tile([128, HID], f32, tag="mmout")
                for k in range(KN):
                    nc.tensor.matmul(ps[:], lhsT=nt[:, k, j * 128:(j + 1) * 128],
                                     rhs=wnode_sb[:, k, :],
                                     start=(k == 0), stop=(k == KN - 1))
                gsl = g_all[:, t * HID:(t + 1) * HID]
                nc.scalar.activation(out=gsl, in_=ps[:], func=AF.Gelu)
                sqj = wkpool.tile([128, HID], f16, tag="sqj")
                nc.vector.scalar_tensor_tensor(
                    out=sqj[:], in0=gsl, scalar=1.0, in1=gsl,
                    op0=OP.mult, op1=OP.mult, accum_out=ex2_n[:, t:t + 1])
                nc.vector.reduce_sum(out=sum_n[:, t:t + 1], in_=gsl, axis=AX.X)

        mu_n = spool.tile([128, NB], f32, tag="mun")
        nc.vector.tensor_scalar(out=mu_n[:], in0=sum_n[:], scalar1=1.0 / HID,
                                scalar2=None, op0=OP.mult)
        nc.vector.tensor_scalar(out=ex2_n[:], in0=ex2_n[:], scalar1=1.0 / HID,
                                scalar2=None, op0=OP.mult)
        tmp_n = spool.tile([128, NB], f32, tag="tmpn")
        nc.vector.scalar_tensor_tensor(out=tmp_n[:], in0=mu_n[:], scalar=-1.0,
                                       in1=mu_n[:], op0=OP.mult, op1=OP.mult)
        var_n = spool.tile([128, NB], f32, tag="varn")
        nc.vector.tensor_tensor(out=var_n[:], in0=tmp_n[:], in1=ex2_n[:], op=OP.add)
        lnv_n = spool.tile([128, NB], f32, tag="lnvn")
        nc.scalar.activation(out=lnv_n[:], in_=var_n[:], func=AF.Ln, bias=EPS)
        rstd_n = spool.tile([128, NB], f32, tag="rstdn")
        nc.scalar.activation(out=rstd_n[:], in_=lnv_n[:], func=AF.Exp, scale=-0.5)
        if meta["node_uni"]:
            rs2_n = spool.tile([128, NB], f32, tag="rs2n")
            nc.vector.tensor_scalar(out=rs2_n[:], in0=rstd_n[:], scalar1=g_nu,
                                    scalar2=None, op0=OP.mult)
            nb_n = spool.tile([128, NB], f32, tag="nbn")
            nc.vector.scalar_tensor_tensor(out=nb_n[:], in0=mu_n[:], scalar=-1.0,
                                           in1=rs2_n[:], op0=OP.mult, op1=OP.mult)
            if b_nu != 0.0:
                nc.vector.tensor_scalar(out=nb_n[:], in0=nb_n[:], scalar1=b_nu,
                                        scalar2=None, op0=OP.add)
        hv_r = hv_in.rearrange("(a p) m -> p a m", p=128)
        for t0 in range(0, NB, NTG):
            tn = min(NTG, NB - t0)
            hv4 = wkpool.tile([128, NTG, HID], f16, tag="hvt")
            for j in range(tn):
                t = t0 + j
                gsl = g_all[:, t * HID:(t + 1) * HID]
                if meta["node_uni"]:
                    nc.vector.tensor_scalar(out=hv4[:, j, :], in0=gsl,
                                            scalar1=rs2_n[:, t:t + 1],
                                            scalar2=nb_n[:, t:t + 1],
                                            op0=OP.mult, op1=OP.add)
                else:
                    zt = wkpool.tile([128, HID], f32, tag="zt")
                    nc.vector.tensor_scalar(out=zt[:], in0=gsl,
                                            scalar1=mu_n[:, t:t + 1],
                                            scalar2=rstd_n[:, t:t + 1],
                                            op0=OP.subtract, op1=OP.mult)
                    nc.vector.tensor_tensor(out=zt[:], in0=zt[:], in1=gnr_sb[:], op=OP.mult)
                    nc.vector.tensor_tensor(out=hv4[:, j, :], in0=zt[:], in1=bnr_sb[:], op=OP.add)
            # Activation-engine HWDGE queue: keeps the hv flush ahead of the
            # pass-1 edge loads the scheduler hoists on the SP queue, so the
            # AllGather starts as soon as the node phase is done.
            nc.scalar.dma_start(out=hv_r[:, t0:t0 + tn, :], in_=hv4[:, :tn, :])

        nc.gpsimd.collective_compute(
            "AllGather", OP.bypass,
            replica_groups=[list(range(cfg.NC))],
            ins=[hv_in[:]], outs=[hv_ag[:]],
        )

        # ============================== phase E pass 1: per-edge q = sum(y'^2)
        # q = x^T (W'W'^T/HID) x on PE+DVE -- no dependence on the AllGather,
        # so the whole stats pass overlaps it.
        GRP = 7
        PB = 2                        # bins per pass-1 DMA pair
        q_all = ppool.tile([128, NB * TPB], f32, tag="q_all")
        for b0 in range(0, NB, PB):
            bn = min(PB, NB - b0)
            eT1 = e1pool.tile([EIN, PB * TPB * 128], f16, tag="eT1")
            nc.sync.dma_start(out=eT1[:, :bn * TPB * 128],
                              in_=edge_T[:, b0 * TPB * 128:(b0 + bn) * TPB * 128])
            xr = e1pool.tile([128, PB * TPB, EIN], f16, tag="xr")
            nc.sync.dma_start(
                out=xr[:, :bn * TPB, :],
                in_=x_rows[:, b0 * TPB * EIN:(b0 + bn) * TPB * EIN])
            for g0 in range(0, bn * TPB, GRP):
                gl = min(GRP, bn * TPB - g0)
                t1g = t1pool.tile([128, GRP, EIN], f32, tag="t1g")
                for j in range(gl):
                    tt = g0 + j
                    nc.tensor.matmul(t1g[:, j, :],
                                     lhsT=eT1[:, tt * 128:(tt + 1) * 128],
                                     rhs=amat_sb[:], start=True, stop=True)
                p2 = wkpool.tile([128, GRP, EIN], f16, tag="p2")
                nc.vector.tensor_tensor(out=p2[:, :gl, :], in0=t1g[:, :gl, :],
                                        in1=xr[:, g0:g0 + gl, :], op=OP.mult)
                nc.vector.reduce_sum(
                    out=q_all[:, b0 * TPB + g0:b0 * TPB + g0 + gl],
                    in_=p2[:, :gl, :], axis=AX.X)

        # batched LN statistics for every edge tile at once (q here is
        # already E[y'^2] because A is pre-divided by HID)
        lnv_e = ppool.tile([128, NB * TPB], f32, tag="lnv_e")
        nc.scalar.activation(out=lnv_e[:], in_=q_all[:], func=AF.Ln, bias=EPS)
        rstd_e = ppool.tile([128, NB * TPB], f32, tag="rstd_e")
        nc.scalar.activation(out=rstd_e[:], in_=lnv_e[:], func=AF.Exp, scale=-0.5)
        if meta["edge_uni"] and g_eu != 1.0:
            nc.vector.tensor_scalar(out=rstd_e[:], in0=rstd_e[:], scalar1=g_eu,
                                    scalar2=None, op0=OP.mult)

        # ====================== phase E pass 2: he, gather, msgs, scatter-sum
        # Wave-split: he production for bin b runs LAG bins ahead of the
        # gather-dependent consumer.  The PE/Act queues then hold LAG bins of
        # AllGather-independent work before the first scatter matmul stalls
        # on the collective, filling the AG window instead of idling.
        h_sb = ppool.tile([128, NB * 128], f32, tag="h_sb")
        last_exp = None
        he_tiles = {}

        eT_pairs = {}

        def produce_he(b):
            nonlocal last_exp
            if b not in eT_pairs:
                b0 = b - (b % 2)
                bn = min(2, NB - b0)
                eTp = wkpool.tile([EIN, 2 * TPB * 128], f16, tag="eT")
                nc.sync.dma_start(out=eTp[:, :bn * TPB * 128],
                                  in_=edge_T[:, b0 * TPB * 128:(b0 + bn) * TPB * 128])
                for j in range(bn):
                    eT_pairs[b0 + j] = (eTp, j)
            eTp, off = eT_pairs.pop(b)
            eT = eTp[:, off * TPB * 128:(off + 1) * TPB * 128]
            he = hepool.tile([128, TPB, HID], f16, tag="he")
            for t in range(TPB):
                hp = pspool.tile([128, HID], f32, tag="mmout")
                nc.tensor.matmul(hp[:], lhsT=eT[:, t * 128:(t + 1) * 128],
                                 rhs=wedge_sb[:], start=True, stop=True)
                col = b * TPB + t
                if meta["edge_uni"]:
                    last_exp = nc.scalar.activation(
                        out=he[:, t, :], in_=hp[:], func=AF.Exp,
                        scale=rstd_e[:, col:col + 1],
                        bias=b_eu if b_eu != 0.0 else 0.0)
                else:
                    zt = wkpool.tile([128, HID], f32, tag="zte")
                    nc.vector.tensor_scalar(out=zt[:], in0=hp[:],
                                            scalar1=rstd_e[:, col:col + 1],
                                            scalar2=None, op0=OP.mult)
                    nc.vector.tensor_tensor(out=zt[:], in0=zt[:], in1=ger_sb[:],
                                            op=OP.mult)
                    nc.vector.tensor_tensor(out=zt[:], in0=zt[:], in1=ber_sb[:],
                                            op=OP.add)
                    last_exp = nc.scalar.activation(out=he[:, t, :], in_=zt[:],
                                                    func=AF.Exp)
            he_tiles[b] = he

        def consume_bin(b):
            gb = gpool.tile([128, TPB, HID], f16, tag="gb")
            col0 = b * TPB * 8
            # dma_gather calls capped at GSTEP*128 indices (HW packet limit).
            def emit_gathers(t_base, ntiles, src_view):
                done = 0
                while done < ntiles:
                    step = min(GSTEP, ntiles - done)
                    nidx = step * 128
                    nc.gpsimd.dma_gather(
                        out_ap=gb[:, t_base + done:t_base + done + step, :],
                        in_ap=src_view,
                        idxs_ap=srcw_sb[:, col0 + (t_base + done) * 8:
                                        col0 + (t_base + done + step) * 8],
                        num_idxs=nidx, num_idxs_reg=nidx, elem_size=HID)
                    done += step
            if K_LO > 0:
                emit_gathers(0, K_LO, hv_ag[0:LO, :])
            if K_HI > 0:
                emit_gathers(K_LO, K_HI, hv_ag[LO:AGROWS, :])

            he = he_tiles.pop(b)
            msgs = wkpool.tile([128, TPB, HID], f16, tag="msgs")
            nc.vector.tensor_tensor(out=msgs[:], in0=he[:], in1=gb[:], op=OP.mult)
            S = wkpool.tile([128, TPB, 128], f16, tag="S")
            dsl = dst_sb[:, b * TPB:(b + 1) * TPB, None].to_broadcast([128, TPB, 128])
            nc.vector.tensor_tensor(out=S[:], in0=iota_sb[:], in1=dsl, op=OP.is_equal)

            hb = hbpool.tile([128, 128], f32, tag="hb")
            for t in range(TPB):
                nc.tensor.matmul(hb[:], lhsT=msgs[:, t, :], rhs=S[:, t, :],
                                 start=(t == 0), stop=(t == TPB - 1))
            nc.vector.tensor_copy(out=h_sb[:, b * 128:(b + 1) * 128], in_=hb[:])

        for i in range(NB + LAG):
            if i < NB:
                produce_he(i)
            if i >= LAG:
                consume_bin(i - LAG)

        # =================================================== phase OUT
        go_all = ppool.tile([128, NB * OUT], f32, tag="go_all")
        ex2_o = spool.tile([128, NB], f32, tag="ex2o")
        sum_o = spool.tile([128, NB], f32, tag="sumo")
        for b in range(NB):
            op_ps = pspool.tile([128, HID], f32, tag="mmout")   # use [:, :OUT]
            nc.tensor.matmul(op_ps[:, :OUT], lhsT=h_sb[:, b * 128:(b + 1) * 128],
                             rhs=wout_sb[:], start=True, stop=True)
            osl = go_all[:, b * OUT:(b + 1) * OUT]
            gelu_i = nc.scalar.activation(out=osl, in_=op_ps[:, :OUT], func=AF.Gelu)
            if last_exp is not None:
                add_dep_helper(gelu_i.ins, last_exp.ins, sync=False,
                               reason="keep OUT-phase gelu after edge-phase exp (ACT tables)")
            sqo = wkpool.tile([128, OUT], f16, tag="sqo")
            nc.vector.scalar_tensor_tensor(
                out=sqo[:], in0=osl, scalar=1.0, in1=osl,
                op0=OP.mult, op1=OP.mult, accum_out=ex2_o[:, b:b + 1])
            nc.vector.reduce_sum(out=sum_o[:, b:b + 1], in_=osl, axis=AX.X)

        mu_o = spool.tile([128, NB], f32, tag="muo")
        nc.vector.tensor_scalar(out=mu_o[:], in0=sum_o[:], scalar1=1.0 / OUT,
                                scalar2=None, op0=OP.mult)
        nc.vector.tensor_scalar(out=ex2_o[:], in0=ex2_o[:], scalar1=1.0 / OUT,
                                scalar2=None, op0=OP.mult)
        tmp_o = spool.tile([128, NB], f32, tag="tmpo")
        nc.vector.scalar_tensor_tensor(out=tmp_o[:], in0=mu_o[:], scalar=-1.0,
                                       in1=mu_o[:], op0=OP.mult, op1=OP.mult)
        var_o = spool.tile([128, NB], f32, tag="varo")
        nc.vector.tensor_tensor(out=var_o[:], in0=tmp_o[:], in1=ex2_o[:], op=OP.add)
        lnv_o = spool.tile([128, NB], f32, tag="lnvo")
        nc.scalar.activation(out=lnv_o[:], in_=var_o[:], func=AF.Ln, bias=EPS)
        rstd_o = spool.tile([128, NB], f32, tag="rstdo")
        nc.scalar.activation(out=rstd_o[:], in_=lnv_o[:], func=AF.Exp, scale=-0.5)
        if meta["out_uni"]:
            rs2_o = spool.tile([128, NB], f32, tag="rs2o")
            nc.vector.tensor_scalar(out=rs2_o[:], in0=rstd_o[:], scalar1=g_ou,
                                    scalar2=None, op0=OP.mult)
            nb_o = spool.tile([128, NB], f32, tag="nbo")
            nc.vector.scalar_tensor_tensor(out=nb_o[:], in0=mu_o[:], scalar=-1.0,
                                           in1=rs2_o[:], op0=OP.mult, op1=OP.mult)
            if b_ou != 0.0:
                nc.vector.tensor_scalar(out=nb_o[:], in0=nb_o[:], scalar1=b_ou,
                                        scalar2=None, op0=OP.add)
        out_all = ppool.tile([128, NB, OUT], f32, tag="out_all")
        for b in range(NB):
            osl = go_all[:, b * OUT:(b + 1) * OUT]
            if meta["out_uni"]:
                nc.vector.tensor_scalar(out=out_all[:, b, :], in0=osl,
                                        scalar1=rs2_o[:, b:b + 1],
                                        scalar2=nb_o[:, b:b + 1],
                                        op0=OP.mult, op1=OP.add)
            else:
                zo = wkpool.tile([128, OUT], f32, tag="zo")
                nc.vector.tensor_scalar(out=zo[:], in0=osl,
                                        scalar1=mu_o[:, b:b + 1],
                                        scalar2=rstd_o[:, b:b + 1],
                                        op0=OP.subtract, op1=OP.mult)
                nc.vector.tensor_tensor(out=zo[:], in0=zo[:], in1=gor_sb[:], op=OP.mult)
                nc.vector.tensor_tensor(out=out_all[:, b, :], in0=zo[:], in1=bor_sb[:],
                                        op=OP.add)
        out_r = out_ext.rearrange("(b p) o -> p b o", p=128)
        nc.sync.dma_start(out=out_r[:], in_=out_all[:])

        for p in (hbpool, t1pool, pspool, gpool, e1pool, ntpool, wkpool,
                  spool, ppool, cpool):
            p.release()

    nc.compile()
    return nc


# ---------------------------------------------------------------- entry point

_CACHE = {}


def _get_program(cfg, meta):
    key = cfg.key() + (meta["node_uni"], meta["edge_uni"], meta["out_uni"],
                       meta["g_nu"], meta["b_nu"], meta["g_eu"], meta["b_eu"],
                       meta["g_ou"], meta["b_ou"])
    if key not in _CACHE:
        _CACHE[key] = build(cfg, meta)
    return _CACHE[key]


def run(cfg, inputs, trace=False, trace_cores=None):
    in_maps, meta = prep(cfg, **inputs)
    nc = _get_program(cfg, meta)
    res = run_bass_kernel_spmd(nc, in_maps, core_ids=list(range(cfg.NC)),
                               trace=trace, trace_cores=trace_cores)
    out = np.empty((cfg.N, cfg.OUT), np.float32)
    for c in range(cfg.NC):
        oc = res.results[c]["out"]
        lo = c * cfg.NPC
        hi = min((c + 1) * cfg.NPC, cfg.N)
        out[lo:hi] = oc[meta["inv"][c]]
    return out, res


def kernel(node_feats, edge_feats, src, dst,
           W_node, g_node, b_node, W_edge, g_edge, b_edge,
           W_out, g_out, b_out):
    cfg = Cfg(n_nodes=node_feats.shape[0], n_edges=edge_feats.shape[0],
              node_in=node_feats.shape[1], edge_in=edge_feats.shape[1],
              hid=W_node.shape[1], out=W_out.shape[1])
    out, _ = run(cfg, dict(
        node_feats=node_feats, edge_feats=edge_feats, src=src, dst=dst,
        W_node=W_node, g_node=g_node, b_node=b_node,
        W_edge=W_edge, g_edge=g_edge, b_edge=b_edge,
        W_out=W_out, g_out=g_out, b_out=b_out))
    return out
